# revision 9
# baseline (speedup 1.0000x reference)
"""Trainium2 Bass kernel for nn_EuclideanAttentionBlock (GNN message passing).

Strategy:
  - Host: sort edges by receiver, shard nodes into 8 contiguous ranges of 4096
    (one per core).  Each core owns the edges whose receiver falls in its
    range, padded so every 128-node group has exactly K 128-edge tiles.
  - Device phase A: every core computes the sender-side projection table
    S = [silu(x@Wk) | silu(x@Wke) | x@Wv]  (fp16, [N, 384]) and the local
    receiver table R = [silu(x@Wq) | silu(x@Wqe)]  (fp16, [4096, 256]).
  - Device phase B: per 1024-edge batch: dma_gather S rows by sender and R
    rows by local receiver, edge-major elementwise attention math, one-hot
    matmul segment-sum into PSUM per 128-node group, flush groups to the
    per-core output slice [4096, 144].  Outputs are disjoint -> host concat.
"""

import sys

for _p in ("/opt/trn_rl_repo", "/root/.axon_site/_ro/pypackages"):
    if _p not in sys.path:
        sys.path.insert(0, _p)

import numpy as np

N = 32768
F = 128
EV = 16
NRBF = 32
NPC = 4096          # nodes per core
G = 32              # 128-node groups per core
NCORES = 8
H = 4
D = 32
ESW = 82            # edge-stream width: rbf 32 | sh 16 | evs 16 | evr 16 | cs 1 | lrecv 1
EV_OFFS = [(0, 1), (1, 3), (4, 5), (9, 7)]   # (offset, width) of degree blocks

_CACHE = {}


class _PhaseAOnly(Exception):
    def __init__(self, nc):
        self.nc = nc


def _build(K):
    import os
    mode = os.environ.get("KMODE", "FULL")
    import concourse.bass as bass
    import concourse.mybir as mybir
    from concourse import bacc, tile

    fp16 = mybir.dt.float16
    f32 = mybir.dt.float32
    i16 = mybir.dt.int16
    AF = mybir.ActivationFunctionType
    ALU = mybir.AluOpType

    Epc = G * K * 128          # edges per core (padded)
    NB2 = Epc // 2048          # gather-pair iterations

    nc = bacc.Bacc(None, target_bir_lowering=False)

    x = nc.declare_dram_parameter("x", [N, F], fp16, isOutput=False)
    evf = nc.declare_dram_parameter("evf", [N, EV], fp16, isOutput=False)
    xl = nc.declare_dram_parameter("xl", [NPC, F], fp16, isOutput=False)
    bdws = nc.declare_dram_parameter("bdws", [F, 384], fp16, isOutput=False)
    bdwr = nc.declare_dram_parameter("bdwr", [F, 256], fp16, isOutput=False)
    wfc = nc.declare_dram_parameter("wfc", [37, 256], fp16, isOutput=False)
    ident = nc.declare_dram_parameter("ident", [128, 128], fp16, isOutput=False)
    iotaf = nc.declare_dram_parameter("iotaf", [128, 128], fp16, isOutput=False)
    es = nc.declare_dram_parameter("es", [Epc, ESW], fp16, isOutput=False)
    lrf = nc.declare_dram_parameter("lrf", [Epc, 1], f32, isOutput=False)
    sidx = nc.declare_dram_parameter("sidx", [128, Epc // 16], i16, isOutput=False)
    rlidx = nc.declare_dram_parameter("rlidx", [128, Epc // 16], i16, isOutput=False)
    dout = nc.declare_dram_parameter("dout", [NPC, 144], f32, isOutput=True)

    stab = nc.dram_tensor("stab", [N, 384], fp16)
    rtab = nc.dram_tensor("rtab", [NPC, 256], fp16)

    with tile.TileContext(nc) as tc:
        with tc.tile_pool(name="pconst", bufs=1) as pc:
            idt = pc.tile([128, 128], fp16)
            nc.sync.dma_start(out=idt[:], in_=ident[:])
            iot = pc.tile([128, 128], fp16)
            nc.sync.dma_start(out=iot[:], in_=iotaf[:])
            wS = pc.tile([128, 384], fp16)
            nc.sync.dma_start(out=wS[:], in_=bdws[:])
            wR = pc.tile([128, 256], fp16)
            nc.sync.dma_start(out=wR[:], in_=bdwr[:])
            wF = pc.tile([37, 256], fp16)
            nc.sync.dma_start(out=wF[:], in_=wfc[:])

            # ---------------- phase A: projection tables ----------------
            with (
                tc.tile_pool(name="pa", bufs=3) as pa,
                tc.tile_pool(name="pap", bufs=2, space="PSUM") as pap,
                tc.tile_pool(name="papz", bufs=1, space="PSUM") as papz,
            ):
                def proj_tiles(src_x, n_rows, w_tile, w_cols, silu_cols, dst,
                               tag):
                    # iterate 4 node-tiles (512 rows) at a time
                    n_it = n_rows // 512
                    for i in range(n_it):
                        x4 = pa.tile([128, 4, F], fp16, tag="x4")
                        nc.sync.dma_start(
                            out=x4[:],
                            in_=src_x[i * 512:(i + 1) * 512].rearrange(
                                "(c p) f -> p c f", p=128),
                        )
                        xT_ps = pap.tile([128, 4, 128], fp16, tag="xT")
                        for c in range(4):
                            nc.tensor.transpose(xT_ps[:, c, :], x4[:, c, :],
                                                idt[:])
                        xT = pa.tile([128, 4, 128], fp16, tag="xTs")
                        nc.vector.tensor_copy(xT[:], xT_ps[:])
                        z_ps = papz.tile([128, 4, 512], f32, tag="z")
                        for c in range(4):
                            nc.tensor.matmul(z_ps[:, c, 0:w_cols], xT[:, c, :],
                                             w_tile[:], start=True, stop=True)
                        s4 = pa.tile([128, 4, w_cols], fp16, tag="s4")
                        nc.scalar.activation(s4[:, :, 0:silu_cols],
                                             z_ps[:, :, 0:silu_cols], AF.Silu)
                        if silu_cols < w_cols:
                            nc.vector.tensor_copy(s4[:, :, silu_cols:w_cols],
                                                  z_ps[:, :, silu_cols:w_cols])
                        nc.sync.dma_start(
                            out=dst[i * 512:(i + 1) * 512].rearrange(
                                "(c p) f -> p c f", p=128),
                            in_=s4[:],
                        )

                proj_tiles(x, N, wS, 384, 256, stab, "s")
                proj_tiles(xl, NPC, wR, 256, 256, rtab, "r")

            # ---------------- phase B: edge processing ----------------
            with (
                tc.tile_pool(name="pb", bufs=2) as pb,
                tc.tile_pool(name="pg", bufs=2) as pg,
                tc.tile_pool(name="psT", bufs=2, space="PSUM") as psT,
                tc.tile_pool(name="psF", bufs=2, space="PSUM") as psF,
                tc.tile_pool(name="psG", bufs=2, space="PSUM") as psG,
            ):
                grp_ps = None
                NB = Epc // 1024
                nb_run = 0 if mode == "A" else NB
                if mode == "A":
                    gs0 = pb.tile([128, 144], f32, tag="gsum")
                    nc.vector.memset(gs0[:], 0.0)
                    for g in range(G):
                        nc.sync.dma_start(
                            out=dout[g * 128:(g + 1) * 128, :], in_=gs0[:])
                for b in range(nb_run):
                    sg = pg.tile([128, 8, 384], fp16, tag="sg")
                    six = pg.tile([128, 64], i16, tag="six")
                    nc.sync.dma_start(out=six[:],
                                      in_=sidx[:, 64 * b:64 * (b + 1)])
                    nc.gpsimd.dma_gather(sg[:], stab[:], six[:], 1024, 1024,
                                         384)
                    rg = pg.tile([128, 8, 256], fp16, tag="rg")
                    rix = pg.tile([128, 64], i16, tag="rix")
                    nc.sync.dma_start(out=rix[:],
                                      in_=rlidx[:, 64 * b:64 * (b + 1)])
                    nc.gpsimd.dma_gather(rg[:], rtab[:], rix[:], 1024, 1024,
                                         256)
                    lr2 = pg.tile([128, 8, 1], f32, tag="lr2")
                    nc.sync.dma_start(
                        out=lr2[:],
                        in_=lrf[1024 * b:1024 * (b + 1)].rearrange(
                            "(c p) q -> p c q", p=128),
                    )
                    es2 = pg.tile([128, 8, ESW], fp16, tag="es2")
                    nc.sync.dma_start(
                        out=es2[:],
                        in_=es[1024 * b:1024 * (b + 1)].rearrange(
                            "(c p) q -> p c q", p=128),
                    )
                    if mode == "AG":
                        gs = pb.tile([128, 144], f32, tag="gsum")
                        nc.vector.tensor_copy(gs[:], sg[:, 0, 0:144])
                        nc.sync.dma_start(
                            out=dout[128 * (b % 32):128 * (b % 32) + 128, :],
                            in_=gs[:])
                        continue
                    if True:
                        sgb = sg[:, :, :]
                        rgb = rg[:, :, :]
                        esb = es2[:, :, :]
                        # ---- feat assembly [rbf | invar | 1] ----
                        feat = pb.tile([128, 8, 37], fp16, tag="feat")
                        nc.vector.tensor_copy(feat[:, :, 0:32],
                                              esb[:, :, 0:32])
                        nc.vector.memset(feat[:, :, 36:37], 1.0)
                        diff = pb.tile([128, 8, EV], fp16, tag="diff")
                        nc.vector.tensor_sub(diff[:], esb[:, :, 48:64],
                                             esb[:, :, 64:80])
                        sq = pb.tile([128, 8, EV], fp16, tag="sq")
                        nc.scalar.square(sq[:], diff[:])
                        with nc.allow_low_precision("fp16 invar blocks"):
                            for j, (off, w) in enumerate(EV_OFFS):
                                if w == 1:
                                    nc.vector.tensor_copy(
                                        feat[:, :, 32 + j:33 + j],
                                        sq[:, :, off:off + 1])
                                else:
                                    nc.vector.tensor_reduce(
                                        feat[:, :, 32 + j:33 + j],
                                        sq[:, :, off:off + w],
                                        axis=mybir.AxisListType.X,
                                        op=ALU.add)
                        # ---- featT + fw = feat @ [Wf_inv | Wf_ev] ----
                        ftT_ps = psT.tile([37, 8, 128], fp16, tag="ftT")
                        for c in range(8):
                            nc.tensor.transpose(ftT_ps[:, c, :], feat[:, c, :],
                                                idt[:])
                        ftT = pb.tile([37, 8, 128], fp16, tag="ftTs")
                        nc.scalar.copy(ftT[:], ftT_ps[:])
                        fw = pb.tile([128, 8, 256], fp16, tag="fw")
                        for hh in range(2):
                            fwp = psF.tile([128, 4, 256], f32, tag="fwp")
                            for c4 in range(4):
                                c = hh * 4 + c4
                                nc.tensor.matmul(fwp[:, c4, :], ftT[:, c, :],
                                                 wF[:], start=True, stop=True)
                            nc.scalar.copy(fw[:, hh * 4:hh * 4 + 4, :], fwp[:])
                        # ---- attention ----
                        qk = pb.tile([128, 8, 256], fp16, tag="qk")
                        nc.vector.tensor_mul(qk[:], rgb[:, :, 0:256],
                                             sgb[:, :, 0:256])
                        qkf = pb.tile([128, 8, 256], fp16, tag="qkf")
                        nc.vector.tensor_mul(qkf[:], qk[:], fw[:])
                        a8 = pb.tile([128, 8, 8], fp16, tag="a8")
                        with nc.allow_low_precision("fp16 head sums"):
                            nc.vector.tensor_reduce(
                                a8[:],
                                qkf[:].rearrange("p c (h d) -> p c h d", d=32),
                                axis=mybir.AxisListType.X,
                                op=ALU.add)
                        acs = pb.tile([128, 8, 8], fp16, tag="acs")
                        cs_b = esb[:, :, 80:81].copy()
                        cs_b.ap = cs_b.ap[:-1] + [[0, 8]]
                        nc.vector.tensor_mul(acs[:], a8[:], cs_b)
                        # ---- messages ----
                        msg = pb.tile([128, 8, 144], fp16, tag="msg")
                        ai = acs[:, :, 0:4].copy()
                        ai.ap = ai.ap + [[0, 32]]
                        nc.vector.tensor_mul(
                            msg[:, :, 0:128].rearrange(
                                "p c (h d) -> p c h d", d=32),
                            sgb[:, :, 256:384].rearrange(
                                "p c (h d) -> p c h d", d=32),
                            ai)
                        for h, (off, w) in enumerate(EV_OFFS):
                            ae = acs[:, :, 4 + h:5 + h].copy()
                            ae.ap = ae.ap[:-1] + [[0, w]]
                            nc.vector.tensor_mul(msg[:, :, 128 + off:128 + off + w],
                                                 esb[:, :, 32 + off:32 + off + w],
                                                 ae)
                        # ---- one-hot scatter into group PSUM ----
                        oh = pb.tile([128, 8, 128], fp16, tag="oh")
                        for c in range(8):
                            nc.vector.tensor_scalar(oh[:, c, :], iot[:],
                                                    lr2[:, c, :], None,
                                                    op0=ALU.is_equal)
                        for c in range(8):
                            t = b * 8 + c
                            g, within = divmod(t, K)
                            if within == 0:
                                grp_ps = psG.tile([128, 144], f32, tag="grp")
                            nc.tensor.matmul(grp_ps[:], oh[:, c, :],
                                             msg[:, c, :],
                                             start=(within == 0),
                                             stop=(within == K - 1),
                                             skip_group_check=True)
                            if within == K - 1:
                                gsb = pb.tile([128, 144], f32, tag="gsb")
                                nc.scalar.copy(gsb[:], grp_ps[:])
                                nc.sync.dma_start(
                                    out=dout[g * 128:(g + 1) * 128, :],
                                    in_=gsb[:])
    nc.finalize()
    return nc


def _wrap_idx(a):
    """edge i -> idxs[i % 16, i // 16], replicated to 128 partitions."""
    w = a.reshape(-1, 16).T
    return np.ascontiguousarray(np.tile(w, (8, 1)))


def _host_prep(inputs):
    inv_features = np.asarray(inputs["inv_features"], np.float32)
    ev_features = np.asarray(inputs["ev_features"], np.float32)
    rbf = np.asarray(inputs["rbf"], np.float32)
    sh_vectors = np.asarray(inputs["sh_vectors"], np.float32)
    cutoffs = np.asarray(inputs["cutoffs"], np.float32)
    senders = np.asarray(inputs["senders"], np.int32)
    receivers = np.asarray(inputs["receivers"], np.int32)
    E = senders.shape[0]

    order = np.argsort(receivers, kind="stable")
    rs = receivers[order]
    ss = senders[order]
    grp = rs >> 7                                  # global 128-node group
    n_grp = (N + 127) // 128
    cnt = np.bincount(grp, minlength=n_grp)
    K = max(1, int(np.ceil(cnt.max() / 128)))
    Epc = G * K * 128

    gstart = np.concatenate([[0], np.cumsum(cnt)[:-1]])
    within = np.arange(E) - gstart[grp]
    core = grp >> 5
    slot = (grp & 31) * (K * 128) + within

    # edge-stream [8, Epc, ESW] fp16
    estream = np.zeros((NCORES, Epc, ESW), np.float16)
    estream[core, slot, 0:32] = rbf[order]
    estream[core, slot, 32:48] = sh_vectors[order]
    estream[core, slot, 48:64] = ev_features[ss]
    estream[core, slot, 64:80] = ev_features[rs]
    estream[core, slot, 80] = (cutoffs[order, 0] / np.sqrt(32.0))
    lrecv = np.zeros((NCORES, Epc, 1), np.float32)
    lrecv[core, slot, 0] = (rs & 127).astype(np.float32)

    sidx_full = np.zeros((NCORES, Epc), np.int16)
    sidx_full[core, slot] = ss.astype(np.int16)
    rlidx_full = np.zeros((NCORES, Epc), np.int16)
    rlidx_full[core, slot] = (rs - core * NPC).astype(np.int16)

    # weights
    W_q_inv = np.asarray(inputs["W_q_inv"], np.float32)
    W_k_inv = np.asarray(inputs["W_k_inv"], np.float32)
    W_v_inv = np.asarray(inputs["W_v_inv"], np.float32)
    W_q_ev = np.asarray(inputs["W_q_ev"], np.float32)
    W_k_ev = np.asarray(inputs["W_k_ev"], np.float32)
    Wf_inv = np.asarray(inputs["Wf_inv"], np.float32)
    bf_inv = np.asarray(inputs["bf_inv"], np.float32)
    Wf_ev = np.asarray(inputs["Wf_ev"], np.float32)
    bf_ev = np.asarray(inputs["bf_ev"], np.float32)

    bdws = np.zeros((F, 384), np.float16)     # [k | ke | v]
    bdwr = np.zeros((F, 256), np.float16)     # [q | qe]
    for h in range(H):
        sl = slice(h * D, (h + 1) * D)
        bdws[sl, 0:128][:, sl] = W_k_inv[h]
        bdws[sl, 128:256][:, sl] = W_k_ev[h]
        bdws[sl, 256:384][:, sl] = W_v_inv[h]
        bdwr[sl, 0:128][:, sl] = W_q_inv[h]
        bdwr[sl, 128:256][:, sl] = W_q_ev[h]

    wfc = np.zeros((37, 256), np.float16)
    wfc[0:36, 0:128] = Wf_inv
    wfc[36, 0:128] = bf_inv
    wfc[0:36, 128:256] = Wf_ev
    wfc[36, 128:256] = bf_ev

    ident = np.eye(128, dtype=np.float16)
    iota = np.tile(np.arange(128, dtype=np.float16), (128, 1))
    x16 = inv_features.astype(np.float16)
    ev16 = ev_features.astype(np.float16)

    in_maps = []
    for c in range(NCORES):
        in_maps.append({
            "x": x16,
            "evf": ev16,
            "xl": x16[c * NPC:(c + 1) * NPC],
            "bdws": bdws,
            "bdwr": bdwr,
            "wfc": wfc,
            "ident": ident,
            "iotaf": iota,
            "es": estream[c],
            "lrf": lrecv[c],
            "sidx": _wrap_idx(sidx_full[c]),
            "rlidx": _wrap_idx(rlidx_full[c]),
        })
    return in_maps, K


def run(inputs, trace=False):
    from concourse.bass_utils import run_bass_kernel_spmd

    in_maps, K = _host_prep(inputs)
    if K not in _CACHE:
        _CACHE[K] = _build(K)
    nc = _CACHE[K]
    res = run_bass_kernel_spmd(nc, in_maps, core_ids=list(range(NCORES)),
                               trace=trace)
    d = np.concatenate([r["dout"] for r in res.results], axis=0)
    d_inv = np.ascontiguousarray(d[:, 0:128])
    d_ev = np.ascontiguousarray(d[:, 128:144])
    return (d_inv, d_ev), res


def kernel(**inputs):
    (d_inv, d_ev), _ = run(inputs, trace=False)
    return d_inv, d_ev
